# revision 2
# baseline (speedup 1.0000x reference)
"""Trainium2 Bass kernel for nn_CausalSelfAttention_2783138808334.

B=8, T=1024, C=64, n_head=1. Data-parallel over batch: one batch per
NeuronCore across 8 cores (weights/tables replicated), gathered on the host.

Per-core algorithm (see emit()):
  qkv = x @ Wqkv.T + b; causal attention with relative-position tables;
  y = (att @ v + attU @ embv) @ Wproj.T + b.

The relative-position gathers reduce to matmuls plus two "skews":
  att2[t,s] = QE[t, t-s]    (QE = q @ embk.T)
  attU[t,u] = att[t, t-u]
Each skew is a DRAM scratch round trip: rows are written (reversed) with
row pitch 2048 and read back with a strided DMA whose partition step is
2047, which makes the diagonal lines contiguous.

v2 changes vs the original kernel:
  - The 72 value-side PE transposes (E.T / attU.T) are replaced by xbar
    DMA-transpose reads straight out of the DRAM scratch: ET[k] comes from
    EDN (E stored forward), EUT[k] comes from EDR (E stored reversed,
    right-aligned at column K0 so the skew stride D-1 is global).
  - att2 is accumulated into the scores by the a2 skew-read itself
    (SWDGE accum_op=add into an SBUF bf16 copy of att1) instead of an
    identity matmul, taking the PE out of the per-tile DMA chain.
  - A PE warm-up burst at t=0 gets HAM to K=8/8 (2.4 GHz) before the real
    matmuls start; previously the whole kernel ran at 1.2 GHz.
  - Engine queues are laid out so no FIFO ever holds an instruction whose
    dependency lands later than the next instruction's (head-of-line):
    DVE does only early-dep PSUM->SBUF copies; ACT owns the exp chain and
    the E-store DMAs; GPSIMD owns the accum read + masks; SYNC owns the
    QED writes and the trailing transpose-reads.
"""
import numpy as np

import concourse.bass as bass
import concourse.bacc as bacc
import concourse.mybir as mybir
from concourse import masks
from concourse.ap import AP

F32 = mybir.dt.float32
BF = mybir.dt.bfloat16
T = 1024
C = 64
NT = 8          # 128-row tiles of T
D = 2048        # scratch DRAM row pitch (elements)
K0 = 1023       # right-align column for EDR rows (reversed E store)
SCALE = 0.125   # 1/sqrt(C)
FILL = -4000.0  # pre-scale mask fill: exp(0.125 * -4000) == 0
N_WARM = 12     # PE warm-up matmuls (HAM needs ~3.4us of sustained activity)


def rev_free(ap):
    """Reverse the (contiguous) free dim of a 2D AP."""
    (ps, pc), (fs, fc) = ap.ap
    assert fs == 1, ap.ap
    return AP(ap.tensor, ap.offset + (fc - 1), [[ps, pc], [-1, fc]])


def mm_chunks(lo, hi, step=512):
    """Split [lo, hi) at 512-element PSUM bank boundaries."""
    a = lo
    while a < hi:
        b = min(hi, (a // step + 1) * step)
        yield a, b
        a = b


def emit(nc, tc, xd, wqkv, bqkv, embk, embv, wproj, bproj, yd):
    with (
        tc.tile_pool(name="const", bufs=1) as cp,
        tc.tile_pool(name="work", bufs=4) as wp,
        tc.tile_pool(name="psum", bufs=1, space="PSUM") as pp,
        tc.tile_pool(name="dram", bufs=1, space="DRAM") as dp,
    ):
        QED = dp.tile([T + 1, D], BF, name="QED").tensor
        EDN = dp.tile([T + 1, D], BF, name="EDN").tensor
        EDR = dp.tile([T + 1, D], BF, name="EDR").tensor

        ident = cp.tile([128, 128], F32)
        masks.make_identity(nc, ident)
        identb = cp.tile([128, 128], BF)
        masks.make_identity(nc, identb)

        # ---- PE warm-up burst (no data deps; drains before real work) ----
        wsrc = cp.tile([128, 512], BF)
        nc.vector.memset(wsrc, 0.0)
        for _ in range(N_WARM):
            pw = pp.tile([128, 512], F32, tag="tp", bufs=2, name="ps_warm")
            nc.tensor.matmul(pw[:, :], identb[:, :], wsrc[:, :],
                             start=True, stop=True)

        # ---- loads (fp32) ----
        X = cp.tile([128, 512], F32)    # x[128n+p, c] at [p, 64n+c]
        EK = cp.tile([128, 512], F32)
        EV = cp.tile([128, 512], F32)
        nc.sync.dma_start(out=X.rearrange("p (n c) -> p n c", c=C),
                          in_=xd.rearrange("(n p) c -> p n c", p=128))
        nc.scalar.dma_start(out=EK.rearrange("p (n c) -> p n c", c=C),
                          in_=embk.rearrange("(n p) c -> p n c", p=128))
        nc.scalar.dma_start(out=EV.rearrange("p (n c) -> p n c", c=C),
                          in_=embv.rearrange("(n p) c -> p n c", p=128))
        W0 = cp.tile([128, C], F32)
        W1 = cp.tile([C, C], F32)
        WP = cp.tile([C, C], F32)
        nc.gpsimd.dma_start(out=W0[:, :], in_=wqkv[0:128, :])
        nc.gpsimd.dma_start(out=W1[:, :], in_=wqkv[128:192, :])
        nc.gpsimd.dma_start(out=WP[:, :], in_=wproj[:, :])
        bq = cp.tile([1, 3 * C], F32)
        bp = cp.tile([1, C], F32)
        nc.gpsimd.dma_start(out=bq[:, :], in_=bqkv.unsqueeze(0))
        nc.gpsimd.dma_start(out=bp[:, :], in_=bproj.unsqueeze(0))
        ones_row = cp.tile([1, T], BF)
        nc.gpsimd.memset(ones_row, 1.0)

        # ---- on-chip transposes + bf16 casts ----
        xT = cp.tile([C, T], BF)
        for n in range(NT):
            ps = pp.tile([C, 128], F32, tag="tp", bufs=2)
            nc.tensor.transpose(ps[:, :], X[:, 64 * n:64 * n + 64], ident[:, :])
            nc.scalar.copy(xT[:, 128 * n:128 * (n + 1)], ps[:, :])
        # KEK: rows 0:64 = embk.T, rows 64:128 = k.T;  qTd: q.T in both halves
        # KEK rows 0:64 hold embk.T with its columns REVERSED, so the QE
        # matmul emits QE row-reversed via a plain (positive-stride) slice.
        KEK = cp.tile([128, T], BF)
        for n in range(NT):
            ps = pp.tile([C, 128], F32, tag="tp", bufs=2)
            nc.tensor.transpose(ps[:, :], EK[:, 64 * n:64 * n + 64], ident[:, :])
            nc.scalar.copy(rev_free(KEK[0:C, T - 128 * (n + 1):T - 128 * n]), ps[:, :])
        WT = cp.tile([C, 3 * C], BF)
        WTq2 = cp.tile([C, 128], BF)    # [Wq.T | Wq.T]
        WTk2 = cp.tile([C, 128], BF)    # [Wk.T | Wk.T]
        bq2 = cp.tile([1, 128], BF)     # [bq | bq]
        bk2 = cp.tile([1, 128], BF)     # [bk | bk]
        ps = pp.tile([C, 128], F32, tag="tp", bufs=2)
        nc.tensor.transpose(ps[:, :], W0[:, :], ident[:, :])
        nc.scalar.copy(WT[:, 0:128], ps[:, :])
        nc.scalar.copy(WTq2[:, 0:C], ps[:, 0:C])
        nc.scalar.copy(WTq2[:, C:128], ps[:, 0:C])
        nc.scalar.copy(WTk2[:, 0:C], ps[:, C:128])
        nc.scalar.copy(WTk2[:, C:128], ps[:, C:128])
        ps = pp.tile([C, 128], F32, tag="tp", bufs=2)
        nc.tensor.transpose(ps[:, 0:C], W1[:, :], ident[0:C, 0:C])
        nc.scalar.copy(WT[:, 128:192], ps[:, 0:C])
        WpT = cp.tile([C, C], F32)
        ps = pp.tile([C, 128], F32, tag="tp", bufs=2)
        nc.tensor.transpose(ps[:, 0:C], WP[:, :], ident[0:C, 0:C])
        nc.vector.tensor_copy(WpT[:, :], ps[:, 0:C])
        EMBV = cp.tile([128, 512], BF)
        nc.vector.tensor_copy(EMBV[:, :], EV[:, :])
        bqb = cp.tile([1, 3 * C], BF)
        nc.vector.tensor_copy(bqb[:, :], bq[:, :])
        nc.vector.tensor_copy(bq2[:, 0:C], bq[:, 0:C])
        nc.vector.tensor_copy(bq2[:, C:128], bq[:, 0:C])
        nc.vector.tensor_copy(bk2[:, 0:C], bq[:, C:128])
        nc.vector.tensor_copy(bk2[:, C:128], bq[:, C:128])

        # ---- qkv projection ----
        # ps_q2: q.T duplicated into both partition halves (col-packed pair);
        # ps_k2: k.T in partitions 64:128.
        ps_q2 = pp.tile([128, T], F32, tag="qe", bufs=1, name="ps_q2")
        ps_k2 = pp.tile([128, T], F32, tag="a1", bufs=1, name="ps_k2")
        for a, b in mm_chunks(0, T):
            nc.tensor.matmul(ps_q2[:, a:b], WTq2[:, :], xT[:, a:b],
                             start=True, stop=False)
            nc.tensor.matmul(ps_k2[:, a:b], WTk2[:, :], xT[:, a:b],
                             start=True, stop=False)
            nc.tensor.matmul(ps_q2[:, a:b], bq2[:, :], ones_row[:, a:b],
                             start=False, stop=True)
            nc.tensor.matmul(ps_k2[:, a:b], bk2[:, :], ones_row[:, a:b],
                             start=False, stop=True)
        qTd = cp.tile([128, T], BF)
        nc.scalar.copy(qTd[:, :], ps_q2[:, :])
        nc.vector.tensor_copy(KEK[C:128, :], ps_k2[C:128, :])
        V = cp.tile([128, 512], BF)     # v[128n+p, c] at [p, 64n+c]
        for n in range(NT):
            ps_v = pp.tile([128, C], F32, tag="tp", bufs=2)
            nc.tensor.matmul(ps_v[:, :], xT[:, 128 * n:128 * (n + 1)], WT[:, 128:192],
                             start=True, stop=False)
            nc.tensor.matmul(ps_v[:, :], ones_row[:, 0:128], bqb[:, 128:192],
                             start=False, stop=True)
            nc.scalar.copy(V[:, 64 * n:64 * (n + 1)], ps_v[:, :])

        # ---- value-side transposed tiles (filled by xbar DMA-transposes) ----
        ET = [cp.tile([128, T], BF, tag=f"et{k}", name=f"et{k}") for k in range(NT)]
        EUT = [cp.tile([128, T], BF, tag=f"eut{k}", name=f"eut{k}") for k in range(NT)]
        for k in range(NT):
            if k % 4 != 0:
                g0 = 512 * (k // 4)
                nc.vector.memset(ET[k][:, g0:128 * k], 0.0)
                nc.vector.memset(EUT[k][:, g0:128 * k], 0.0)

        EN = [cp.tile([128, T], BF, tag=f"en{i}", name=f"en{i}") for i in range(NT)]
        Zc = cp.tile([128, NT], F32)
        rz = cp.tile([128, NT], F32)

        # ---- main pipeline over t-tiles (i = 7..0) ----
        # Per tile: row-packed score matmuls; QE (cast bf16) -> QED rows
        # [1..1024]; a1 -> bf16 SBUF; the reversed-skew a2 readback ACCUMULATES
        # into it (SWDGE CCE add); mask the diagonal block; exp (Z via
        # accum_out) -> EN; EN -> EDN (forward) and, reversed, -> EDR
        # right-aligned at K0; xbar transpose-reads then deliver
        # ET[i] = E.T column block and EUT[i] = attU.T column block.
        for i in range(NT - 1, -1, -1):
            Wd = 128 * (i + 1)          # triangular: only d,s <= t needed
            i0 = 128 * i
            nt = T - i0
            ps_qe = pp.tile([128, T], F32, tag="qe", bufs=1, name="ps_qe")
            ps_a1 = pp.tile([128, T], F32, tag="a1", bufs=1, name="ps_a1")
            qeb = wp.tile([128, T], BF, tag="qeb")
            a1s = wp.tile([128, T], BF, tag="a1s")
            for a, b in mm_chunks(0, Wd):
                nc.tensor.matmul(ps_qe[:, a:b], qTd[0:C, i0:i0 + 128],
                                 KEK[0:C, T - Wd + a:T - Wd + b], start=True, stop=True)
                nc.tensor.matmul(ps_a1[:, a:b], qTd[C:128, i0:i0 + 128],
                                 KEK[C:128, a:b], start=True, stop=True)
                nc.vector.tensor_copy(qeb[:, a:b], ps_qe[:, a:b])
                nc.vector.tensor_copy(a1s[:, a:b], ps_a1[:, a:b])
            # rows shifted +1 so the skew read never underflows the buffer
            nc.sync.dma_start(out=AP(QED, (i0 + 1) * D, [[D, 128], [1, Wd]]),
                              in_=qeb[:, 0:Wd])
            # a1s[p, s] += QE[t, t-s] (normal s order; contiguous inner stride)
            nc.gpsimd.dma_start(out=a1s[:, 0:Wd],
                                in_=AP(QED, (i0 + 1) * D + Wd - 1 - i0,
                                       [[D - 1, 128], [1, Wd]]),
                                accum_op=mybir.AluOpType.add)
            # garbage/mask region s > t lives entirely in the last 128 cols
            nc.gpsimd.affine_select(out=a1s[:, Wd - 128:Wd], in_=a1s[:, Wd - 128:Wd],
                                    pattern=[[-1, 128]],
                                    compare_op=mybir.AluOpType.is_ge, fill=FILL,
                                    base=0, channel_multiplier=1)
            nc.scalar.activation(EN[i][:, 0:Wd], a1s[:, 0:Wd],
                                 mybir.ActivationFunctionType.Exp, scale=SCALE,
                                 accum_out=Zc[:, i:i + 1])
            enr = wp.tile([128, T], BF, tag="enr")
            nc.scalar.copy(enr[:, 0:Wd], rev_free(EN[i][:, 0:Wd]))
            # E stores: forward rows (for E.T reads) and reversed rows
            # right-aligned at column K0 (for attU.T skew reads).
            nc.scalar.dma_start(out=AP(EDN, (i0 + 1) * D, [[D, 128], [1, Wd]]),
                                in_=EN[i][:, 0:Wd])
            nc.scalar.dma_start(out=AP(EDR, (i0 + 1) * D + K0 - (Wd - 1),
                                       [[D, 128], [1, Wd]]),
                                in_=enr[:, 0:Wd])
            # ET[i][s, t] = E[t, s] for t in [i0, T): plain transpose read.
            nc.sync.dma_start(out=ET[i][:, i0:T],
                              in_=AP(EDN, (i0 + 1) * D + i0, [[D, nt], [1, 128]]),
                              transpose=True)
            # EUT[i][u, t] = E[t, t-u]: EDR flat addr (t+1)*D + K0 - t + u.
            nc.sync.dma_start(out=EUT[i][:, i0:T],
                              in_=AP(EDR, (i0 + 1) * D + K0,
                                     [[D - 1, nt], [1, 128]]),
                              transpose=True)
            # diagonal block: zero where u > t (reads row-tail garbage there)
            nc.gpsimd.affine_select(out=EUT[i][:, i0:i0 + 128],
                                    in_=EUT[i][:, i0:i0 + 128],
                                    pattern=[[1, 128]],
                                    compare_op=mybir.AluOpType.is_ge, fill=0.0,
                                    base=0, channel_multiplier=-1)
        nc.vector.reciprocal(rz[:, :], Zc[:, :])

        # ---- value matmuls (k descending: ET/EUT[k] arrive in that order) ----
        ps_y1 = pp.tile([C, 512], F32, tag="y1", bufs=1, name="ps_y1")
        ps_y0 = pp.tile([C, 512], F32, tag="y0", bufs=1, name="ps_y0")
        for k in range(NT - 1, -1, -1):
            nc.tensor.matmul(ps_y1[:, :], V[:, 64 * k:64 * (k + 1)],
                             ET[k][:, 512:1024], start=(k == NT - 1), stop=False)
            nc.tensor.matmul(ps_y1[:, :], EMBV[:, 64 * k:64 * (k + 1)],
                             EUT[k][:, 512:1024], start=False, stop=(k == 0))
            if k <= 3:
                nc.tensor.matmul(ps_y0[:, :], V[:, 64 * k:64 * (k + 1)],
                                 ET[k][:, 0:512], start=(k == 3), stop=False)
                nc.tensor.matmul(ps_y0[:, :], EMBV[:, 64 * k:64 * (k + 1)],
                                 EUT[k][:, 0:512], start=False, stop=(k == 0))
        ysT = cp.tile([C, T], F32)
        nc.scalar.copy(ysT[:, 512:1024], ps_y1[:, :])
        nc.scalar.copy(ysT[:, 0:512], ps_y0[:, :])

        # ---- output projection; bias enters as Z[t]*bproj so the final 1/Z
        # scale leaves it intact ----
        Zrow = cp.tile([1, T], F32)
        for i in range(NT):
            ps_zr = pp.tile([1, 128], F32, tag="tp", bufs=2, name="ps_zr")
            nc.tensor.matmul(ps_zr[:, :], Zc[:, i:i + 1], ident[:, :],
                             start=True, stop=True)
            nc.vector.tensor_copy(Zrow[:, 128 * i:128 * (i + 1)], ps_zr[:, :])
        Y = cp.tile([128, 512], F32)    # y[128n+p, c] at [p, 64n+c]
        for i in range(NT):
            ps_p = pp.tile([128, C], F32, tag="tp", bufs=2, name="ps_p")
            nc.tensor.matmul(ps_p[:, :], ysT[:, 128 * i:128 * (i + 1)], WpT[:, :],
                             start=True, stop=False)
            nc.tensor.matmul(ps_p[:, :], Zrow[:, 128 * i:128 * (i + 1)], bp[:, :],
                             start=False, stop=True)
            nc.vector.tensor_scalar_mul(Y[:, 64 * i:64 * (i + 1)], ps_p[:, :],
                                        rz[:, i:i + 1])
        nc.sync.dma_start(out=yd.rearrange("(n p) c -> p n c", p=128),
                          in_=Y.rearrange("p (n c) -> p n c", c=C))


_NC_CACHE = None


def _build():
    global _NC_CACHE
    if _NC_CACHE is not None:
        return _NC_CACHE
    nc = bacc.Bacc("TRN2", target_bir_lowering=False, debug=False)
    xd = nc.dram_tensor("x", [T, C], F32, kind="ExternalInput")
    wqkv = nc.dram_tensor("Wqkv", [3 * C, C], F32, kind="ExternalInput")
    bqkv = nc.dram_tensor("bqkv", [3 * C], F32, kind="ExternalInput")
    embk = nc.dram_tensor("embk", [T, C], F32, kind="ExternalInput")
    embv = nc.dram_tensor("embv", [T, C], F32, kind="ExternalInput")
    wproj = nc.dram_tensor("Wproj", [C, C], F32, kind="ExternalInput")
    bproj = nc.dram_tensor("bproj", [C], F32, kind="ExternalInput")
    yd = nc.dram_tensor("y", [T, C], F32, kind="ExternalOutput")
    from concourse.tile import TileContext
    with TileContext(nc) as tc:
        emit(nc, tc, xd.ap(), wqkv.ap(), bqkv.ap(), embk.ap(), embv.ap(),
             wproj.ap(), bproj.ap(), yd.ap())
    nc.compile()
    _NC_CACHE = nc
    return nc


def run_spmd(inputs, **kwargs):
    from concourse.bass_utils import run_bass_kernel_spmd
    x = np.asarray(inputs["x"], dtype=np.float32)
    B = x.shape[0]
    nc = _build()
    shared = {k: np.ascontiguousarray(np.asarray(inputs[k], dtype=np.float32))
              for k in ("Wqkv", "bqkv", "embk", "embv", "Wproj", "bproj")}
    in_maps = [dict(shared, x=np.ascontiguousarray(x[b])) for b in range(B)]
    res = run_bass_kernel_spmd(nc, in_maps, core_ids=list(range(B)), **kwargs)
    y = np.stack([r["y"] for r in res.results], axis=0)
    return y, res


def kernel(**inputs):
    y, _ = run_spmd(inputs)
    return y
